# revision 37
# baseline (speedup 1.0000x reference)
"""Multi-head attention block (B=4, S=2048, D=1024, H=16) on 8 TRN2 cores.

Sharding: data-parallel over batch (4 batches x 2 cores) and tensor-parallel
over heads (8 heads per core).  Each core computes, for its (batch, head-group):
  Q^T/K^T (head-dim-major) and V (seq-major) projections, causal attention
  (scores transposed: S^T[k,q] = K Q^T, exp without max-subtraction, row-sum
  via an appended ones-column in the PV matmul), context, and a partial output
  projection with its w_o column slice.  The host sums the two partial outputs
  per batch (the "all-reduce after w_o") and adds b_o.

Schedule: a single pool scope with no phase barriers.  The priority
scheduler keeps the ACT engine (exp over ~S^2/2 causal score elements, the
per-pair pacemaker) saturated while PE idle time inside each pair's
attention window is filled with the next pair's projections (interleaved
as fill thunks), the V projection, and — in the last pair — the output
projection of already-finished query rows.  Causal trimming skips
score/exp columns strictly below the diagonal (never read by PV); only
the 128-wide diagonal sub-blocks get a triangular mask (on the otherwise
idle GPSIMD engine, into separate band tiles).  Wide [128, 1024] PSUM
score tiles halve the per-instruction exp overhead; a sub-chunk pipeline
(2 score groups of lookahead before the previous chunk's PV) caps live
score-tile count.  Outputs are written bf16 (summed in f32 on the host).

Matmuls run in bf16 (1 PE cycle/row vs fp32's 4); accumulation is always
fp32 in PSUM.  kernel(**inputs) takes full unsharded inputs and returns
the full output.  TimelineSim estimate: 241808 ns (baseline was 357544).
"""

import numpy as np

import concourse.bass as bass
import concourse.mybir as mybir
import concourse.tile as tile
from concourse import bacc
from concourse.bass_utils import run_bass_kernel_spmd
from concourse.masks import make_identity

B, S, D, H = 4, 2048, 1024, 16
DK = D // H            # 64 head dim
P = 128                # partitions
NCORES = 8
HPC = H // 2           # 8 heads per core
DPC = HPC * DK         # 512 projected dims per core
NPAIR = DPC // P       # 4 head-pairs per core
KT = D // P            # 8 contraction tiles for projections
SC_W = 512             # phase-1 seq chunk width
N_SC = S // SC_W
QC_W = 512             # phase-2 query chunk width
N_QC = S // QC_W
NKB = S // P           # 16 key blocks
F32 = mybir.dt.float32
BF16 = mybir.dt.bfloat16
F32R = mybir.dt.float32r

_NC_CACHE: dict = {}


def _build_nc(causal: bool, reps: int = 1, mmdt: str = "bf16", phases: int = 3) -> bass.Bass:
    """mmdt: 'bf16' (default, 1 cyc/row), 'f32r' (1 cyc/row, flaky on HW),
    or 'f32' (4 cyc/row, exact)."""
    mm_dt = {"bf16": BF16, "f32r": F32R, "f32": F32}[mmdt]
    # bf16: host ships pre-converted bf16 inputs -> DMA directly, no converts.
    # f32r: DMA f32 then round on DVE (verifier requires a rounding producer).
    in_dt = BF16 if mmdt == "bf16" else F32
    needs_cvt = mmdt == "f32r" 

    def mm(out, lhsT, rhs, **kw):
        if mmdt == "f32r":
            lhsT = lhsT.bitcast(F32R)
            rhs = rhs.bitcast(F32R)
        nc.tensor.matmul(out, lhsT=lhsT, rhs=rhs, **kw)

    nc = bacc.Bacc(
        "TRN2",
        debug=False,
        enable_asserts=False,
        target_bir_lowering=False,
        num_devices=NCORES,
    )

    qT = nc.dram_tensor("qT", [D, S], in_dt, kind="ExternalInput").ap()
    kT = nc.dram_tensor("kT", [D, S], in_dt, kind="ExternalInput").ap()
    vT = nc.dram_tensor("vT", [D, S], in_dt, kind="ExternalInput").ap()
    wqT = nc.dram_tensor("wqT", [D, DPC], in_dt, kind="ExternalInput").ap()
    wkT = nc.dram_tensor("wkT", [D, DPC], in_dt, kind="ExternalInput").ap()
    wvT = nc.dram_tensor("wvT", [D, DPC], in_dt, kind="ExternalInput").ap()
    woT = nc.dram_tensor("woT", [DPC, D], in_dt, kind="ExternalInput").ap()
    bq = nc.dram_tensor("bq", [DPC], F32, kind="ExternalInput").ap()
    bk = nc.dram_tensor("bk", [DPC], F32, kind="ExternalInput").ap()
    bv = nc.dram_tensor("bv", [DPC], F32, kind="ExternalInput").ap()
    out = nc.dram_tensor("out", [S, D], F32, kind="ExternalOutput").ap()

    from contextlib import ExitStack

    with tile.TileContext(nc) as tc, ExitStack() as octx:
        if reps > 1:
            octx.enter_context(tc.For_i(0, reps, 1))
        ctx = octx.enter_context(ExitStack())
        singles = ctx.enter_context(tc.tile_pool(name="singles", bufs=1))

        identity = singles.tile([P, P], mm_dt if mmdt == "bf16" else F32)
        make_identity(nc, identity)

        if causal:
            # tril[k, u] = 1.0 if u >= k else 0.0 — the mask for the 128x128
            # diagonal sub-block of any query chunk (all other sub-blocks are
            # either fully valid or never read by PV).
            tril = singles.tile([P, P], mm_dt if mmdt == "bf16" else F32)
            nc.gpsimd.memset(tril, 1.0)
            nc.gpsimd.affine_select(
                out=tril,
                in_=tril,
                compare_op=mybir.AluOpType.is_ge,
                fill=0.0,
                base=0,
                channel_multiplier=-1,
                pattern=[[1, P]],
            )

        bq_sb = singles.tile([P, NPAIR], F32)
        bk_sb = singles.tile([P, NPAIR], F32)
        bv_sb = singles.tile([P, NPAIR], F32)

        def load_biases():
            nc.sync.dma_start(bq_sb, bq.rearrange("(pair p) -> p pair", p=P))
            nc.sync.dma_start(bk_sb, bk.rearrange("(pair p) -> p pair", p=P))
            nc.sync.dma_start(bv_sb, bv.rearrange("(pair p) -> p pair", p=P))

        woT_sb = singles.tile([P, NPAIR, D], mm_dt)

        # Persistent activations.  Per-pair (and per-seq-block for V)
        # tiles keep cross-phase dependency tracking exact, so the
        # scheduler can overlap pair p+1's projections with pair p's
        # attention without false WAR serialization.
        QT_p = [singles.tile([P, S], mm_dt, name=f"QT{p}") for p in range(NPAIR)]
        KT_p = [singles.tile([P, S], mm_dt, name=f"KT{p}") for p in range(NPAIR)]
        ctxT_p = [singles.tile([P, S], mm_dt, name=f"ctxT{p}") for p in range(NPAIR)]
        V_b = [singles.tile([P, HPC, DK + 1], mm_dt, name=f"V{b}") for b in range(NKB)]
        for b in range(NKB):
            nc.gpsimd.memset(V_b[b][:, :, DK:DK + 1], 1.0)

        # ---- Unified schedule ----
        # One pool scope for projections, attention, and the output
        # projection: no phase barriers, so the priority scheduler fills
        # PE idle time during the ACT(exp)-bound attention of pair p with
        # the projection matmuls of pair p+1 (emitted just after att(p)).
        # The output projection for finished q-rows is emitted inside the
        # last pair's attention.
        NQB = QC_W // P   # 128-row query sub-blocks per chunk
        NDC = D // 512
        assert not needs_cvt, "f32r input conversion path was removed"

        xst = ctx.enter_context(tc.tile_pool(name="xstage", bufs=4))
        ost = ctx.enter_context(tc.tile_pool(name="ostage", bufs=3))
        ptp = ctx.enter_context(tc.tile_pool(name="ptpool", bufs=28))
        btp = ctx.enter_context(tc.tile_pool(name="bandpool", bufs=16))
        stg = ctx.enter_context(tc.tile_pool(name="stage", bufs=2))
        lit = ctx.enter_context(tc.tile_pool(name="little", bufs=8))
        # PSUM budget: sp 2x(2 banks) + cp 2x(1) + pp 2x(1) = 8 banks.
        # pp slots (2KB) are shared by projection, ctx-transpose, and
        # output-projection tiles via a common tag.
        pp = ctx.enter_context(tc.tile_pool(name="ppsum", bufs=2, space="PSUM"))
        sp = ctx.enter_context(tc.tile_pool(name="spsum", bufs=2, space="PSUM"))
        cp = ctx.enter_context(tc.tile_pool(name="cpsum", bufs=2, space="PSUM"))

        # weight tiles are declared here but their DMAs are emitted at the
        # point of first use (after the first q/k activation chunks) so the
        # serial DMA queue delivers score inputs as early as possible.
        w_sbs = [singles.tile([P, KT, DPC], mm_dt, name=f"w_sb{which}")
                 for which in range(3)]
        w_drams = [wqT, wkT, wvT]

        xrs = [x.rearrange("(kt p) s -> p kt s", p=P) for x in (qT, kT, vT)]

        def load_w(which, pair=None):
            # pair=None loads the whole weight; otherwise just that pair's
            # 128-column slice (0.36us) so the first projection can start
            # without waiting for the full 2.9us weight DMA.
            wr = w_drams[which].rearrange("(kt p) d -> p kt d", p=P)
            if pair is None:
                nc.sync.dma_start(w_sbs[which], wr)
            else:
                csl = slice(pair * P, (pair + 1) * P)
                nc.sync.dma_start(w_sbs[which][:, :, csl], wr[:, :, csl])

        def qk_chunk(pairs, which, sc):
            # one x-chunk DMA feeds the projections of every pair in
            # `pairs` (halves HBM re-reads vs per-pair streaming).
            bias_sb = bq_sb if which == 0 else bk_sb
            w_sb = w_sbs[which]
            x_sb = xst.tile([P, KT, SC_W], in_dt, name="x_sb")
            nc.sync.dma_start(x_sb, xrs[which][:, :, sc * SC_W:(sc + 1) * SC_W])
            for pair in pairs:
                dest = (QT_p if which == 0 else KT_p)[pair]
                ps = pp.tile([P, SC_W], F32, name="ps_p1", tag="ps_p1")
                for kt in range(KT):
                    mm(
                        ps,
                        w_sb[:, kt, pair * P:(pair + 1) * P],
                        x_sb[:, kt, :],
                        start=(kt == 0),
                        stop=(kt == KT - 1),
                    )
                nc.vector.tensor_scalar_add(
                    out=dest[:, sc * SC_W:(sc + 1) * SC_W],
                    in0=ps,
                    scalar1=bias_sb[:, pair:pair + 1],
                )

        def v_chunk(sc):
            w_sb = w_sbs[2]
            x_sb = xst.tile([P, KT, SC_W], in_dt, name="x_sb")
            nc.sync.dma_start(x_sb, xrs[2][:, :, sc * SC_W:(sc + 1) * SC_W])
            for ss in range(SC_W // P):
                ps = pp.tile([P, DPC], F32, name="ps_p1", tag="ps_p1")
                for kt in range(KT):
                    mm(
                        ps,
                        x_sb[:, kt, ss * P:(ss + 1) * P],
                        w_sb[:, kt, :],
                        start=(kt == 0),
                        stop=(kt == KT - 1),
                    )
                sblk = sc * (SC_W // P) + ss
                nc.vector.tensor_copy(
                    out=V_b[sblk][:, :, 0:DK],
                    in_=ps.rearrange("p (h d) -> p h d", h=HPC),
                )

        def proj_first():
            # pair-0 weight slices and the sc0 q/k chunks go first on the
            # serial DMA queue so the first score matmul can issue early.
            load_w(0, 0)
            load_w(1, 0)
            load_biases()
            for sc in range(N_SC):
                qk_chunk((0,), 0, sc)
                qk_chunk((0,), 1, sc)
            load_w(2)
            v_chunk(0)



        def ph3_sb(sb, wide=False):
            # output projection for q-rows [sb*P, (sb+1)*P): contract the
            # four pairs' ctx^T slices against their w_o rows.  In the tail
            # (after the last scores) the wide score-psum tiles are free:
            # one [P, 1024] tile covers both output column halves, with a
            # single copy and one contiguous row DMA.
            if wide:
                ps = sp.tile([P, 2 * SC_W], F32, name="ps_s")
                for dmc in range(NDC):
                    for pair in range(NPAIR):
                        mm(
                            ps[:, dmc * 512:(dmc + 1) * 512],
                            ctxT_p[pair][:, sb * P:(sb + 1) * P],
                            woT_sb[:, pair, dmc * 512:(dmc + 1) * 512],
                            start=(pair == 0),
                            stop=(pair == NPAIR - 1),
                        )
                o_sb = ost.tile([P, 2 * SC_W], F32, name="o_sbw", tag="o_sb")
                nc.scalar.copy(out=o_sb, in_=ps)
                nc.sync.dma_start(out[sb * P:(sb + 1) * P, :], o_sb)
                return
            for dmc in range(NDC):
                ps = pp.tile([P, SC_W], F32, name="ps_p1", tag="ps_p1")
                for pair in range(NPAIR):
                    mm(
                        ps,
                        ctxT_p[pair][:, sb * P:(sb + 1) * P],
                        woT_sb[:, pair, dmc * 512:(dmc + 1) * 512],
                        start=(pair == 0),
                        stop=(pair == NPAIR - 1),
                    )
                o_sb = ost.tile([P, 2 * SC_W], F32, name="o_sbw", tag="o_sb")
                nc.vector.tensor_copy(out=o_sb[:, 0:SC_W], in_=ps)
                nc.sync.dma_start(
                    out[sb * P:(sb + 1) * P, dmc * 512:(dmc + 1) * 512],
                    o_sb[:, 0:SC_W]
                )

        # transpose scratch shares pp's 2KB slots (bf16: 1024 cols,
        # f32: 512 cols); only the first P columns are written.
        TPW = 8 * P if mm_dt != F32 else 4 * P

        def att(pair, fills=()):
            fills = list(fills)

            def run_fills(n):
                while n > 0 and fills:
                    fills.pop(0)()
                    n -= 1

            ctx_stage = stg.tile([P, NKB, P], mm_dt if mmdt == "bf16" else F32, name="ctx_stage")

            def emit_st_pair(j, pts, kbg_lo, kbg_hi):
                # Two heads per pair; per head, consecutive k-blocks go to
                # adjacent halves of a wide PSUM tile so one wide exp
                # covers both (halving per-instruction ACT init overhead).
                # The heads' lhsT partition bases are 0 and 64, so their
                # matmuls land in different PE row groups (row tiling).
                #
                # Causal trimming: for a diagonal k-block (c = kb - j*NQB
                # >= 0) the first c*P query columns lie strictly below the
                # diagonal and are never read by emit_pv (which only takes
                # kb <= qb, i.e. qq >= c) — skip them in the score matmul
                # and start the exp at the left block's offset (the unread
                # gap in the middle exps stale PSUM; its output columns
                # are never consumed).  Only the 128-wide diagonal
                # sub-blocks need the triangular mask (applied on Pool).
                psl0, psl1 = slice(0, DK), slice(DK, 2 * DK)
                kb_hi = min(NKB, (j + 1) * NQB) if causal else NKB
                diag = pts[2]
                hp_ctx = tc.high_priority()
                hp_ctx.__enter__()
                for kbg in range(kbg_lo, min(kbg_hi, kb_hi // 2)):
                    kb0 = 2 * kbg
                    for hp, psl in ((0, psl0), (1, psl1)):
                        ps = sp.tile([P, 2 * QC_W], F32, name="ps_s")
                        ptt = ptp.tile([P, 2 * QC_W], mm_dt, name="pt")
                        offs = []
                        for half, kb in enumerate((kb0, kb0 + 1)):
                            c = kb - j * NQB if causal and kb >= j * NQB else -1
                            off = c * P if c > 0 else 0
                            offs.append((c, off))
                            qcols = slice(j * QC_W + off, (j + 1) * QC_W)
                            hbase = half * QC_W
                            mm(
                                ps[:, hbase + off:hbase + QC_W],
                                KT_p[pair][psl, kb * P:(kb + 1) * P],
                                QT_p[pair][psl, qcols],
                                start=True,
                                stop=True,
                            )
                            pts[hp][kb] = ptt[:, half * QC_W:(half + 1) * QC_W]
                        off0 = offs[0][1]
                        nc.scalar.activation(
                            ptt[:, off0:], ps[:, off0:],
                            mybir.ActivationFunctionType.Exp,
                            scale=1.0 / np.sqrt(DK),
                        )
                        for half, (c, off) in enumerate(offs):
                            if c >= 0:
                                # masked diagonal band goes to its own tile
                                # (read by PV for qq == c) so the exp tiles
                                # never enter a write-after-read chain with
                                # the Pool engine.
                                band = slice(half * QC_W + c * P,
                                             half * QC_W + (c + 1) * P)
                                bt = btp.tile([P, P], mm_dt, name="bt")
                                nc.gpsimd.tensor_mul(
                                    bt, ptt[:, band], tril
                                )
                                diag[(hp, kb0 + half)] = bt
                hp_ctx.__exit__(None, None, None)

            def emit_pv_pair(j, pts):
                # interleave the two heads per q-block: one head's PV
                # matmuls overlap the other's reciprocal/normalize chain.
                for qq in range(NQB):
                    qb = j * NQB + qq
                    kmax = (qb + 1) if causal else NKB
                    for hp in (0, 1):
                        h = pair * 2 + hp
                        psl = slice(hp * DK, (hp + 1) * DK)
                        cps = cp.tile([P, DK + 1], F32, name="cps")
                        for kb in range(kmax):
                            if causal and kb == qb:
                                lhsT = pts[2][(hp, kb)]
                            else:
                                lhsT = pts[hp][kb][:, qq * P:(qq + 1) * P]
                            nc.tensor.matmul(
                                cps,
                                lhsT=lhsT,
                                rhs=V_b[kb][:, h, :],
                                start=(kb == 0),
                                stop=(kb == kmax - 1),
                            )
                        recip = lit.tile([P, 1], F32, name="recip")
                        nc.vector.reciprocal(recip, cps[:, DK:DK + 1])
                        nc.vector.tensor_scalar_mul(
                            ctx_stage[:, qb, psl], cps[:, 0:DK], scalar1=recip
                        )

            def finish_chunk(jj):
                # both heads' PV for chunk jj are emitted: transpose those
                # ctx columns to head-major (+v-bias); for the last pair
                # the output projection of the finished q-rows follows.
                for sb in range(jj * NQB, (jj + 1) * NQB):
                    tps = pp.tile([P, TPW], mm_dt, name="tps", tag="ps_p1")
                    nc.tensor.transpose(tps[:, 0:P], ctx_stage[:, sb, :], identity)
                    nc.vector.tensor_scalar_add(
                        out=ctxT_p[pair][:, sb * P:(sb + 1) * P],
                        in0=tps[:, 0:P],
                        scalar1=bv_sb[:, pair:pair + 1],
                    )
                    if pair == NPAIR - 1:
                        ph3_sb(sb, wide=(jj == N_QC - 1))

            # sub-chunk pipeline: only two score groups of chunk j are
            # emitted before chunk j-1's PV — enough exp backlog to cover
            # the PV's PE time while capping live score-tile count (the
            # pt pool gates the next pair's scores via slot recycling).
            LOOKAHEAD = 2
            prev = None
            for j in range(N_QC):
                cur = ({}, {}, {})
                emit_st_pair(j, cur, 0, LOOKAHEAD)
                if prev is not None:
                    emit_pv_pair(j - 1, prev)
                    finish_chunk(j - 1)
                emit_st_pair(j, cur, LOOKAHEAD, NKB)
                run_fills(4)
                prev = cur
            emit_pv_pair(N_QC - 1, prev)
            finish_chunk(N_QC - 1)
            run_fills(len(fills))

        # Emission order == scheduler priority.
        def qk_fills(pairs):
            return [
                (lambda w=w, sc=sc: qk_chunk(pairs, w, sc))
                for sc in range(N_SC) for w in (0, 1)
            ]

        def load_woT():
            nc.sync.dma_start(
                woT_sb, woT.rearrange("(pair p) dm -> p pair dm", p=P))

        proj_first()
        if phases >= 2:
            f1 = qk_fills((1,))
            fills0 = ([lambda: load_w(0, 1), lambda: load_w(1, 1),
                       lambda: v_chunk(1)] + f1[0:2]
                      + [lambda: v_chunk(2)] + f1[2:4]
                      + [lambda: v_chunk(3)] + f1[4:6]
                      + [load_woT] + f1[6:8])
            att(0, fills0)
            att(1, [lambda: load_w(0, 2), lambda: load_w(1, 2)] + qk_fills((2,)))
            att(2, [lambda: load_w(0, 3), lambda: load_w(1, 3)] + qk_fills((3,)))
            att(3)
        else:
            v_chunk(1)
            v_chunk(2)
            v_chunk(3)
            load_woT()
            load_w(0)
            load_w(1)
            for pair in range(1, NPAIR):
                for sc in range(N_SC):
                    qk_chunk((pair,), 0, sc)
                    qk_chunk((pair,), 1, sc)

    if not nc.is_finalized():
        nc.finalize()
    return nc


def _get_nc(causal: bool, reps: int = 1, **kw) -> bass.Bass:
    key = (causal, reps, tuple(sorted(kw.items())))
    if key not in _NC_CACHE:
        _NC_CACHE[key] = _build_nc(causal, reps, **kw)
    return _NC_CACHE[key]


def _make_in_maps(q, k, v, w_q, w_k, w_v, w_o, b_q, b_k, b_v, in_np=None):
    import ml_dtypes
    if in_np is None:
        in_np = ml_dtypes.bfloat16
    in_maps = []
    qb = [np.ascontiguousarray(q[b].T.astype(in_np)) for b in range(B)]
    kb = [np.ascontiguousarray(k[b].T.astype(in_np)) for b in range(B)]
    vb = [np.ascontiguousarray(v[b].T.astype(in_np)) for b in range(B)]
    for c in range(NCORES):
        b, g = divmod(c, 2)
        hsl = slice(g * DPC, (g + 1) * DPC)
        in_maps.append({
            "qT": qb[b],
            "kT": kb[b],
            "vT": vb[b],
            "wqT": np.ascontiguousarray(w_q[hsl, :].T.astype(in_np)),
            "wkT": np.ascontiguousarray(w_k[hsl, :].T.astype(in_np)),
            "wvT": np.ascontiguousarray(w_v[hsl, :].T.astype(in_np)),
            "woT": np.ascontiguousarray(w_o[:, hsl].T.astype(in_np)),
            "bq": np.ascontiguousarray(b_q[hsl]),
            "bk": np.ascontiguousarray(b_k[hsl]),
            "bv": np.ascontiguousarray(b_v[hsl]),
        })
    return in_maps


def kernel(q, k, v, mask, w_q, b_q, w_k, b_k, w_v, b_v, w_o, b_o, **run_kwargs):
    q = np.asarray(q, np.float32)
    k = np.asarray(k, np.float32)
    v = np.asarray(v, np.float32)
    w_q = np.asarray(w_q, np.float32)
    w_k = np.asarray(w_k, np.float32)
    w_v = np.asarray(w_v, np.float32)
    w_o = np.asarray(w_o, np.float32)
    b_q = np.asarray(b_q, np.float32)
    b_k = np.asarray(b_k, np.float32)
    b_v = np.asarray(b_v, np.float32)
    b_o = np.asarray(b_o, np.float32)

    mask_b = np.asarray(mask).reshape(S, S).astype(bool)
    causal = bool(np.array_equal(mask_b, np.tril(np.ones((S, S), bool))))
    if not causal:
        assert mask_b.all(), "only causal or all-ones masks are supported"

    nc = _get_nc(causal)
    in_maps = _make_in_maps(q, k, v, w_q, w_k, w_v, w_o, b_q, b_k, b_v)

    res = run_bass_kernel_spmd(nc, in_maps, core_ids=list(range(NCORES)), **run_kwargs)
    outs = [np.asarray(r["out"], dtype=np.float32) for r in res.results]
    full = np.stack(
        [outs[2 * b] + outs[2 * b + 1] + b_o[None, :] for b in range(B)]
    ).astype(np.float32)
    kernel.last_result = res
    return full


kernel.last_result = None

